# revision 19
# baseline (speedup 1.0000x reference)
"""Trainium2 Bass kernel for nn_AttentionSortNet (sparse_attention).

Per bh slice (data-parallel over bh across 8 cores):
  b_q = bucket-mean(q), b_k = bucket-mean(k)          (64 buckets x 128 elems)
  sq = b_q + q_pos, sk = b_k + k_pos
  R  = sq @ sk^T                                       (64 x 64)
  E0 = exp((ln(relu(R)+eps) + gumbel) / T)
  8x Sinkhorn in prob domain: E /= rowsum; E /= colsum
  out = E

v2 design (per core, 4 bh = 2 bh-pairs):
  - q/k staged as bf16 on the host (rel err ~1.5e-3 << 2e-2 budget),
    halving HBM traffic; all big loads are HWDGE on the sync queue
    (no SWDGE -> no gpsimd descriptor-gen/drain overhead).
  - bucket means: per (pair, tensor) 4 uneven chunks; chunk0 -> DVE
    strided reduce (offloads PE below the DMA rate), chunks 1-3 -> PE
    accumulating identity matmuls; small last chunk shortens the tail.
  - pos embeddings are host-prescaled by 128 so the mean finish is
    one fused tensor_tensor_reduce (scale 1/128).
  - E0 = exp(ln(relu R + eps)/T) * exp(g/T), with exp(g/T) computed
    during the DMA window (also warms the Exp table).
  - Sinkhorn without transposes: row-divide (DVE) -> column sums via a
    block-diag ones matmul replicated across partitions (PE) -> fused
    col-divide + next-iteration row sums via tensor_tensor_reduce.
    Packing: E[128=(v,i), (h,j)] holds bh = 2h+v.
  - single output DMA through a rearranged DRAM access pattern.
"""
import sys

sys.path.insert(0, "/opt/trn_rl_repo")

import numpy as np
import ml_dtypes

import concourse.bass as bass
import concourse.bacc as bacc
import concourse.mybir as mybir
from concourse import tile
from concourse.bass_utils import run_bass_kernel_spmd

HEADS = 8
BUCKETS = 64
DIM = 64
TEMP = 0.7
EPS = 1e-6
N_CORES = 8
BH = 32
SEQ = 8192
NBH = BH // N_CORES        # 4 bh per core
PAIRS = NBH // 2           # 2 bh-pairs per core
SINKHORN_ITER = 8
# per (pair, tensor): seq elems per partition = 8192; chunk0 goes to DVE,
# chunks 1-3 to PE; sizes in elems/partition (multiples of 512 = ri*d)
CHUNK_SZ = (1024, 3072, 3584, 512)
RI = 8                     # ri lanes accumulated in PSUM, reduced on DVE

F32 = mybir.dt.float32
F32R = mybir.dt.float32r
BF16 = mybir.dt.bfloat16
AF = mybir.ActivationFunctionType
AX = mybir.AxisListType
ALU = mybir.AluOpType


def _build_program():
    nc = bacc.Bacc("TRN2", target_bir_lowering=False, debug=False, num_devices=N_CORES)

    q_d = nc.dram_tensor("q", [NBH, SEQ, DIM], BF16, kind="ExternalInput")
    k_d = nc.dram_tensor("k", [NBH, SEQ, DIM], BF16, kind="ExternalInput")
    # pre-stacked on host: [128=(v,row), pair, 64]; pos is pre-scaled by 128
    qp_d = nc.dram_tensor("posq", [128, PAIRS, DIM], F32, kind="ExternalInput")
    kp_d = nc.dram_tensor("posk", [128, PAIRS, DIM], F32, kind="ExternalInput")
    g_d = nc.dram_tensor("gum", [128, PAIRS, BUCKETS], F32, kind="ExternalInput")
    eyeb_d = nc.dram_tensor("eyeb", [128, 128], BF16, kind="ExternalInput")
    eye_d = nc.dram_tensor("eye", [128, 128], F32, kind="ExternalInput")
    w_d = nc.dram_tensor("wones", [128, 128], F32, kind="ExternalInput")
    out_d = nc.dram_tensor("out", [NBH, BUCKETS, BUCKETS], F32, kind="ExternalOutput")

    with tile.TileContext(nc) as tc:
        with (
            tc.tile_pool(name="const", bufs=1) as constp,
            tc.tile_pool(name="data", bufs=8) as datap,
            tc.tile_pool(name="work", bufs=3) as workp,
            tc.tile_pool(name="small", bufs=4) as smallp,
            tc.tile_pool(name="persist", bufs=1) as persistp,
            tc.tile_pool(name="epool", bufs=3) as ep,
            tc.tile_pool(name="pacc", bufs=2, space=bass.MemorySpace.PSUM) as pacc,
            tc.tile_pool(name="ptr", bufs=2, space=bass.MemorySpace.PSUM) as ptr,
            tc.tile_pool(name="pR", bufs=2, space=bass.MemorySpace.PSUM) as pR,
            tc.tile_pool(name="pcsr", bufs=2, space=bass.MemorySpace.PSUM) as pcsr,
        ):
            # small loads on the sync HWDGE queue (big q/k chunks ride the
            # gpsimd SWDGE queue; scalar queue stays free for activations)
            eyeb = constp.tile([128, 128], BF16, tag="eyeb")
            nc.sync.dma_start(eyeb[:], eyeb_d[:])
            gum = constp.tile([128, PAIRS, BUCKETS], F32, tag="gum")
            nc.sync.dma_start(gum[:], g_d[:])
            posq = constp.tile([128, PAIRS, DIM], F32, tag="posq")
            nc.sync.dma_start(posq[:], qp_d[:])
            posk = constp.tile([128, PAIRS, DIM], F32, tag="posk")
            nc.sync.dma_start(posk[:], kp_d[:])
            eye = constp.tile([128, 128], F32, tag="eye")
            nc.sync.dma_start(eye[:], eye_d[:])
            wones = constp.tile([128, 128], F32R, tag="wones")
            nc.gpsimd.dma_start(wones[:], w_d[:])

            # Ln table warm during the DMA window (Exp is warmed by the
            # preloaded combined table; see _preload_act_table)
            tw = constp.tile([128, 1], F32, tag="tw")
            nc.vector.memset(tw[:], 1.0)
            nc.scalar.activation(tw[:], tw[:], AF.Ln)

            seed = persistp.tile([128, PAIRS], F32, tag="seed")
            E0 = ep.tile([128, 2 * BUCKETS], F32, tag="E")

            for pi in range(PAIRS):
                sT = {}
                for nm, src, pos in (("q", q_d, posq), ("k", k_d, posk)):
                    # [2, 8192, 64] -> [128=(bh,bu), 8192]: partition line is
                    # the full bucket (128 seq rows x 64 d), contiguous.
                    view = src[2 * pi : 2 * pi + 2].rearrange(
                        "b (bu sl) d -> (b bu) (sl d)", bu=BUCKETS, sl=SEQ // BUCKETS
                    )
                    # SWDGE: the HW DGE tops out ~90 GB/s generating
                    # descriptors; gpsimd software DGE sustains ~400 GB/s
                    chunks = []
                    off = 0
                    for ci, csz in enumerate(CHUNK_SZ):
                        ch = datap.tile([128, csz], BF16, tag="data")
                        # chunk0 rides the (otherwise idle) sync HWDGE queue,
                        # relieving the SWDGE descriptor generator
                        eng = nc.sync if ci == 0 else nc.gpsimd
                        eng.dma_start(ch[:], view[:, off : off + csz])
                        chunks.append(ch)
                        off += csz

                    # chunk0 on DVE: strided reduce over seq rows, then
                    # pp = partial/128 + pos (all off the critical tail)
                    part = smallp.tile([128, DIM], F32, tag="part")
                    nc.vector.reduce_sum(
                        part[:],
                        chunks[0][:].rearrange(
                            "p (rl d) -> p d rl", rl=CHUNK_SZ[0] // DIM, d=DIM
                        ),
                        axis=AX.X,
                    )
                    pp = smallp.tile([128, DIM], F32, tag="pp")
                    nc.vector.tensor_scalar(
                        out=pp[:], in0=part[:], scalar1=1.0 / 128.0,
                        scalar2=None, op0=ALU.mult,
                    )
                    nc.vector.tensor_add(pp[:], pp[:], pos[:, pi, :])

                    # chunks 1-3 on PE: accumulating identity matmuls with
                    # CONTIGUOUS 512-elem moving slices (strided bf16 moving
                    # runs ~2x slower on PE)
                    acc = pacc.tile([128, DIM * RI], F32, tag="acc")
                    n_mm = [csz // (DIM * RI) for csz in CHUNK_SZ[1:]]
                    total_mm = sum(n_mm)
                    mm_i = 0
                    for ci, ch in enumerate(chunks[1:]):
                        for j in range(n_mm[ci]):
                            nc.tensor.matmul(
                                acc[:],
                                eyeb[:],
                                ch[:, j * DIM * RI : (j + 1) * DIM * RI],
                                start=(mm_i == 0),
                                stop=(mm_i == total_mm - 1),
                            )
                            mm_i += 1

                    # finish: s = acc-mean (eyeb = eye/128) + pp; acc holds
                    # (ri, d) interleaved, reduce ri via strided view
                    red = workp.tile([128, DIM], F32, tag="red")
                    nc.vector.reduce_sum(
                        red[:],
                        acc[:].rearrange("p (ri d) -> p d ri", ri=RI, d=DIM),
                        axis=AX.X,
                    )
                    s_sb = workp.tile([128, DIM], F32, tag="s")
                    nc.vector.tensor_tensor(
                        out=s_sb[:], in0=red[:], in1=pp[:], op=ALU.add
                    )

                    # transpose to [64 d, 128 (v,row)] for the R contraction
                    tps = ptr.tile([64, 128], F32, tag="tp")
                    nc.tensor.transpose(tps[:], s_sb[:], eye[:])
                    t_sb = persistp.tile([64, 128], F32, tag=f"sT{nm}{pi}")
                    nc.vector.tensor_copy(t_sb[:], tps[:])
                    sT[nm] = t_sb

                # R[i, j] = sum_d sq[i, d] sk[j, d]; bh pair stacked on partitions
                Rps = pR.tile([128, BUCKETS], F32, tag="R")
                for v in range(2):
                    nc.tensor.matmul(
                        Rps[64 * v : 64 * (v + 1), :],
                        sT["q"][:, 64 * v : 64 * (v + 1)],
                        sT["k"][:, 64 * v : 64 * (v + 1)],
                        start=True,
                        stop=True,
                    )

                # E0 column strip h=pi: exp((ln(relu R + eps) + g)/T), with
                # row sums accumulated by the ACT engine as the Sinkhorn seed.
                y = workp.tile([128, BUCKETS], F32, tag="y")
                nc.vector.tensor_scalar(
                    out=y[:], in0=Rps[:], scalar1=0.0, scalar2=EPS,
                    op0=ALU.max, op1=ALU.add,
                )
                u = workp.tile([128, BUCKETS], F32, tag="u")
                nc.scalar.activation(u[:], y[:], AF.Ln)
                nc.vector.tensor_add(u[:], u[:], gum[:, pi, :])
                nc.scalar.activation(
                    E0[:, 64 * pi : 64 * (pi + 1)], u[:], AF.Exp,
                    scale=1.0 / TEMP,
                    accum_out=seed[:, pi : pi + 1],
                )

            # Sinkhorn, prob domain, no transposes. E [128=(v,i), (h,j)],
            # bh = 2h+v. Per iteration: row-divide, block colsum matmul
            # (replicated across partitions), fused col-divide + row sums.
            cur, rs = E0, seed
            for it in range(SINKHORN_ITER):
                rsi = smallp.tile([128, PAIRS], F32, tag="rsi")
                nc.vector.reciprocal(rsi[:], rs[:])
                E1 = ep.tile([128, 2 * BUCKETS], F32R, tag="E1")
                nc.vector.tensor_tensor(
                    out=E1[:].rearrange("p (h j) -> p h j", h=2),
                    in0=cur[:].rearrange("p (h j) -> p h j", h=2),
                    in1=rsi[:].unsqueeze(-1).broadcast_to((128, PAIRS, BUCKETS)),
                    op=ALU.mult,
                )
                csr = pcsr.tile([128, 2 * BUCKETS], F32, tag="csr")
                nc.tensor.matmul(csr[:], wones[:], E1[:], start=True, stop=True)
                csri = workp.tile([128, 2 * BUCKETS], F32, tag="csri")
                nc.vector.reciprocal_approx_fast(csri[:], csr[:])
                nxt = ep.tile([128, 2 * BUCKETS], F32, tag="E")
                nc.vector.tensor_tensor(
                    out=nxt[:], in0=E1[:], in1=csri[:], op=ALU.mult
                )
                cur = nxt
                if it < SINKHORN_ITER - 1:
                    rs2 = smallp.tile([128, PAIRS], F32, tag="rs")
                    nc.vector.reduce_sum(
                        rs2[:], nxt[:].rearrange("p (h j) -> p h j", h=2), axis=AX.X
                    )
                    rs = rs2

            # output: out[2h+v][i, j] = E[(v,i), (h,j)]; one DMA per
            # partition half v on separate queues
            ov = out_d[:].rearrange("(h v) i j -> v i h j", h=2, v=2)
            nc.sync.dma_start(
                ov[0], cur[0:64].rearrange("p (h j) -> p h j", h=2)
            )
            nc.scalar.dma_start(
                ov[1], cur[64:128].rearrange("p (h j) -> p h j", h=2)
            )

    _preload_act_table(nc)
    nc.compile()
    return nc


# act_info.json act_func_sets index of natural_log_exp_and_others, the one
# table that serves Ln AND Exp (and Relu/Copy). Pre-loading it up front
# makes Bacc's membership-based fixpoint skip every per-activation
# ACT_TABLE_LOAD (1.28us each), two of which would land on the critical
# tail between pair-1's Ln and Exp.
ACT_SET_LN_EXP = 6


def _preload_act_table(nc, set_id=ACT_SET_LN_EXP):
    load = mybir.InstLoadActFuncSet(
        name=nc.get_next_instruction_name(), act_func_set_id=set_id, ins=[], outs=[]
    )
    seen_act_engine = False
    for blk in nc.main_func.blocks:
        for idx, inst in enumerate(blk.instructions):
            eng = getattr(inst, "engine", None)
            if eng != mybir.EngineType.Activation:
                continue
            # skip the framework preamble (branches/barriers); insert at the
            # first Activation-engine instruction of the kernel body
            if isinstance(
                inst,
                (
                    mybir.InstDMACopy,
                    mybir.InstActivation,
                    mybir.InstLoadActFuncSet,
                ),
            ):
                load.engine = eng
                nc.register_instruction(load)
                blk.instructions.insert(idx, load)
                return
            seen_act_engine = True
    raise AssertionError("no activation-engine body instruction found")


_NC = None


def _get_program():
    global _NC
    if _NC is None:
        _NC = _build_program()
    return _NC


def _stack_pairs(a):
    # [4, X, T] (bh-major) -> [128=(v,X), pair, T] with bh = 2*pair + v
    x, t = a.shape[1], a.shape[2]
    return np.ascontiguousarray(
        a.reshape(PAIRS, 2, x, t).transpose(1, 2, 0, 3).reshape(2 * x, PAIRS, t)
    )


def _make_in_maps(inputs):
    q = np.asarray(inputs["q"], dtype=np.float32).astype(ml_dtypes.bfloat16)
    k = np.asarray(inputs["k"], dtype=np.float32).astype(ml_dtypes.bfloat16)
    qpe = np.asarray(inputs["q_pos_emb"], dtype=np.float32)
    kpe = np.asarray(inputs["k_pos_emb"], dtype=np.float32)
    g = np.ascontiguousarray(inputs["gumbel"], dtype=np.float32)

    b = BH // HEADS
    qpos = np.broadcast_to(qpe, (b, HEADS, BUCKETS, DIM)).reshape(BH, BUCKETS, DIM)
    kpos = np.broadcast_to(kpe, (b, HEADS, BUCKETS, DIM)).reshape(BH, BUCKETS, DIM)
    eye = np.eye(128, dtype=np.float32)
    eyeb = (np.eye(128, dtype=np.float32) / 128.0).astype(ml_dtypes.bfloat16)
    wones = np.kron(np.eye(2, dtype=np.float32), np.ones((64, 64), np.float32))

    in_maps = []
    for c in range(N_CORES):
        sl = slice(NBH * c, NBH * (c + 1))
        in_maps.append(
            {
                "q": np.ascontiguousarray(q[sl]),
                "k": np.ascontiguousarray(k[sl]),
                "posq": _stack_pairs(qpos[sl]),
                "posk": _stack_pairs(kpos[sl]),
                "gum": _stack_pairs(g[sl]),
                "eyeb": eyeb,
                "eye": eye,
                "wones": wones,
            }
        )
    return in_maps


def run(inputs, trace=False):
    nc = _get_program()
    in_maps = _make_in_maps(inputs)
    res = run_bass_kernel_spmd(
        nc, in_maps, core_ids=list(range(N_CORES)), trace=trace
    )
    out = np.concatenate(
        [res.results[c]["out"] for c in range(N_CORES)], axis=0
    ).astype(np.float32)
    return out, res


def kernel(**inputs) -> np.ndarray:
    out, _ = run(inputs, trace=False)
    return out


# revision 20
# speedup vs baseline: 1.0929x; 1.0929x over previous
"""Trainium2 Bass kernel for nn_AttentionSortNet (sparse_attention).

Per bh slice (data-parallel over bh across 8 cores):
  b_q = bucket-mean(q), b_k = bucket-mean(k)          (64 buckets x 128 elems)
  sq = b_q + q_pos, sk = b_k + k_pos
  R  = sq @ sk^T                                       (64 x 64)
  E0 = exp((ln(relu(R)+eps) + gumbel) / T)
  8x Sinkhorn in prob domain: E /= rowsum; E /= colsum
  out = E

v2 design (per core, 4 bh = 2 bh-pairs):
  - q/k staged as bf16 on the host (rel err ~1.5e-3 << 2e-2 budget),
    halving HBM traffic; all big loads are HWDGE on the sync queue
    (no SWDGE -> no gpsimd descriptor-gen/drain overhead).
  - bucket means: per (pair, tensor) 4 uneven chunks; chunk0 -> DVE
    strided reduce (offloads PE below the DMA rate), chunks 1-3 -> PE
    accumulating identity matmuls; small last chunk shortens the tail.
  - pos embeddings are host-prescaled by 128 so the mean finish is
    one fused tensor_tensor_reduce (scale 1/128).
  - E0 = exp(ln(relu R + eps)/T) * exp(g/T), with exp(g/T) computed
    during the DMA window (also warms the Exp table).
  - Sinkhorn without transposes: row-divide (DVE) -> column sums via a
    block-diag ones matmul replicated across partitions (PE) -> fused
    col-divide + next-iteration row sums via tensor_tensor_reduce.
    Packing: E[128=(v,i), (h,j)] holds bh = 2h+v.
  - single output DMA through a rearranged DRAM access pattern.
"""
import sys

sys.path.insert(0, "/opt/trn_rl_repo")

import numpy as np
import ml_dtypes

import concourse.bass as bass
import concourse.bacc as bacc
import concourse.mybir as mybir
from concourse import tile
from concourse.bass_utils import run_bass_kernel_spmd

HEADS = 8
BUCKETS = 64
DIM = 64
TEMP = 0.7
EPS = 1e-6
N_CORES = 8
BH = 32
SEQ = 8192
NBH = BH // N_CORES        # 4 bh per core
PAIRS = NBH // 2           # 2 bh-pairs per core
SINKHORN_ITER = 8
# per (pair, tensor): seq elems per partition = 8192; chunk0 goes to DVE,
# chunks 1-3 to PE; sizes in elems/partition (multiples of 512 = ri*d)
CHUNK_SZ = (1024, 3072, 3072, 1024)
RI = 8                     # ri lanes accumulated in PSUM, reduced on DVE

F32 = mybir.dt.float32
F32R = mybir.dt.float32r
BF16 = mybir.dt.bfloat16
AF = mybir.ActivationFunctionType
AX = mybir.AxisListType
ALU = mybir.AluOpType


def _build_program():
    nc = bacc.Bacc("TRN2", target_bir_lowering=False, debug=False, num_devices=N_CORES)

    q_d = nc.dram_tensor("q", [NBH, SEQ, DIM], BF16, kind="ExternalInput")
    k_d = nc.dram_tensor("k", [NBH, SEQ, DIM], BF16, kind="ExternalInput")
    # pre-stacked on host: [128=(v,row), pair, 64]; pos is pre-scaled by 128
    qp_d = nc.dram_tensor("posq", [128, PAIRS, DIM], F32, kind="ExternalInput")
    kp_d = nc.dram_tensor("posk", [128, PAIRS, DIM], F32, kind="ExternalInput")
    g_d = nc.dram_tensor("gum", [128, PAIRS, BUCKETS], F32, kind="ExternalInput")
    eyeb_d = nc.dram_tensor("eyeb", [128, 128], BF16, kind="ExternalInput")
    eye_d = nc.dram_tensor("eye", [128, 128], F32, kind="ExternalInput")
    w_d = nc.dram_tensor("wones", [128, 128], F32, kind="ExternalInput")
    out_d = nc.dram_tensor("out", [NBH, BUCKETS, BUCKETS], F32, kind="ExternalOutput")

    with tile.TileContext(nc) as tc:
        with (
            tc.tile_pool(name="const", bufs=1) as constp,
            tc.tile_pool(name="data", bufs=8) as datap,
            tc.tile_pool(name="work", bufs=3) as workp,
            tc.tile_pool(name="small", bufs=4) as smallp,
            tc.tile_pool(name="persist", bufs=1) as persistp,
            tc.tile_pool(name="epool", bufs=3) as ep,
            tc.tile_pool(name="pacc", bufs=2, space=bass.MemorySpace.PSUM) as pacc,
            tc.tile_pool(name="ptr", bufs=2, space=bass.MemorySpace.PSUM) as ptr,
            tc.tile_pool(name="pR", bufs=2, space=bass.MemorySpace.PSUM) as pR,
            tc.tile_pool(name="pcsr", bufs=2, space=bass.MemorySpace.PSUM) as pcsr,
        ):
            # small loads on the sync HWDGE queue (big q/k chunks ride the
            # gpsimd SWDGE queue; scalar queue stays free for activations)
            eyeb = constp.tile([128, 128], BF16, tag="eyeb")
            nc.sync.dma_start(eyeb[:], eyeb_d[:])
            gum = constp.tile([128, PAIRS, BUCKETS], F32, tag="gum")
            nc.sync.dma_start(gum[:], g_d[:])
            posq = constp.tile([128, PAIRS, DIM], F32, tag="posq")
            nc.sync.dma_start(posq[:], qp_d[:])
            posk = constp.tile([128, PAIRS, DIM], F32, tag="posk")
            nc.sync.dma_start(posk[:], kp_d[:])
            eye = constp.tile([128, 128], F32, tag="eye")
            nc.sync.dma_start(eye[:], eye_d[:])
            wones = constp.tile([128, 128], F32R, tag="wones")
            nc.gpsimd.dma_start(wones[:], w_d[:])

            # Ln table warm during the DMA window (Exp is warmed by the
            # preloaded combined table; see _preload_act_table)
            tw = constp.tile([128, 1], F32, tag="tw")
            nc.vector.memset(tw[:], 1.0)
            nc.scalar.activation(tw[:], tw[:], AF.Ln)

            seed = persistp.tile([128, PAIRS], F32, tag="seed")
            E0 = ep.tile([128, 2 * BUCKETS], F32, tag="E")

            for pi in range(PAIRS):
                sT = {}
                for nm, src, pos in (("q", q_d, posq), ("k", k_d, posk)):
                    # [2, 8192, 64] -> [128=(bh,bu), 8192]: partition line is
                    # the full bucket (128 seq rows x 64 d), contiguous.
                    view = src[2 * pi : 2 * pi + 2].rearrange(
                        "b (bu sl) d -> (b bu) (sl d)", bu=BUCKETS, sl=SEQ // BUCKETS
                    )
                    # SWDGE: the HW DGE tops out ~90 GB/s generating
                    # descriptors; gpsimd software DGE sustains ~400 GB/s
                    chunks = []
                    off = 0
                    for csz in CHUNK_SZ:
                        ch = datap.tile([128, csz], BF16, tag="data")
                        nc.gpsimd.dma_start(ch[:], view[:, off : off + csz])
                        chunks.append(ch)
                        off += csz

                    # chunk0 on DVE: strided reduce over seq rows, then
                    # pp = partial/128 + pos (all off the critical tail)
                    part = smallp.tile([128, DIM], F32, tag="part")
                    nc.vector.reduce_sum(
                        part[:],
                        chunks[0][:].rearrange(
                            "p (rl d) -> p d rl", rl=CHUNK_SZ[0] // DIM, d=DIM
                        ),
                        axis=AX.X,
                    )
                    pp = smallp.tile([128, DIM], F32, tag="pp")
                    nc.vector.tensor_scalar(
                        out=pp[:], in0=part[:], scalar1=1.0 / 128.0,
                        scalar2=None, op0=ALU.mult,
                    )
                    nc.vector.tensor_add(pp[:], pp[:], pos[:, pi, :])

                    # chunks 1-3 on PE: accumulating identity matmuls with
                    # CONTIGUOUS 512-elem moving slices (strided bf16 moving
                    # runs ~2x slower on PE)
                    acc = pacc.tile([128, DIM * RI], F32, tag="acc")
                    n_mm = [csz // (DIM * RI) for csz in CHUNK_SZ[1:]]
                    total_mm = sum(n_mm)
                    mm_i = 0
                    for ci, ch in enumerate(chunks[1:]):
                        for j in range(n_mm[ci]):
                            nc.tensor.matmul(
                                acc[:],
                                eyeb[:],
                                ch[:, j * DIM * RI : (j + 1) * DIM * RI],
                                start=(mm_i == 0),
                                stop=(mm_i == total_mm - 1),
                            )
                            mm_i += 1

                    # finish: s = acc-mean (eyeb = eye/128) + pp; acc holds
                    # (ri, d) interleaved, reduce ri via strided view
                    red = workp.tile([128, DIM], F32, tag="red")
                    nc.vector.reduce_sum(
                        red[:],
                        acc[:].rearrange("p (ri d) -> p d ri", ri=RI, d=DIM),
                        axis=AX.X,
                    )
                    s_sb = workp.tile([128, DIM], F32, tag="s")
                    nc.vector.tensor_tensor(
                        out=s_sb[:], in0=red[:], in1=pp[:], op=ALU.add
                    )

                    # transpose to [64 d, 128 (v,row)] for the R contraction
                    tps = ptr.tile([64, 128], F32, tag="tp")
                    nc.tensor.transpose(tps[:], s_sb[:], eye[:])
                    t_sb = persistp.tile([64, 128], F32, tag=f"sT{nm}{pi}")
                    nc.vector.tensor_copy(t_sb[:], tps[:])
                    sT[nm] = t_sb

                # R[i, j] = sum_d sq[i, d] sk[j, d]; bh pair stacked on partitions
                Rps = pR.tile([128, BUCKETS], F32, tag="R")
                for v in range(2):
                    nc.tensor.matmul(
                        Rps[64 * v : 64 * (v + 1), :],
                        sT["q"][:, 64 * v : 64 * (v + 1)],
                        sT["k"][:, 64 * v : 64 * (v + 1)],
                        start=True,
                        stop=True,
                    )

                # E0 column strip h=pi: exp((ln(relu R + eps) + g)/T), with
                # row sums accumulated by the ACT engine as the Sinkhorn seed.
                y = workp.tile([128, BUCKETS], F32, tag="y")
                nc.vector.tensor_scalar(
                    out=y[:], in0=Rps[:], scalar1=0.0, scalar2=EPS,
                    op0=ALU.max, op1=ALU.add,
                )
                u = workp.tile([128, BUCKETS], F32, tag="u")
                nc.scalar.activation(u[:], y[:], AF.Ln)
                nc.vector.tensor_add(u[:], u[:], gum[:, pi, :])
                nc.scalar.activation(
                    E0[:, 64 * pi : 64 * (pi + 1)], u[:], AF.Exp,
                    scale=1.0 / TEMP,
                    accum_out=seed[:, pi : pi + 1],
                )

            # Sinkhorn, prob domain, no transposes. E [128=(v,i), (h,j)],
            # bh = 2h+v. Per iteration: row-divide, block colsum matmul
            # (replicated across partitions), fused col-divide + row sums.
            cur, rs = E0, seed
            for it in range(SINKHORN_ITER):
                rsi = smallp.tile([128, PAIRS], F32, tag="rsi")
                nc.vector.reciprocal(rsi[:], rs[:])
                E1 = ep.tile([128, 2 * BUCKETS], F32R, tag="E1")
                nc.vector.tensor_tensor(
                    out=E1[:].rearrange("p (h j) -> p h j", h=2),
                    in0=cur[:].rearrange("p (h j) -> p h j", h=2),
                    in1=rsi[:].unsqueeze(-1).broadcast_to((128, PAIRS, BUCKETS)),
                    op=ALU.mult,
                )
                csr = pcsr.tile([128, 2 * BUCKETS], F32, tag="csr")
                nc.tensor.matmul(csr[:], wones[:], E1[:], start=True, stop=True)
                csri = workp.tile([128, 2 * BUCKETS], F32, tag="csri")
                nc.vector.reciprocal_approx_fast(csri[:], csr[:])
                nxt = ep.tile([128, 2 * BUCKETS], F32, tag="E")
                nc.vector.tensor_tensor(
                    out=nxt[:], in0=E1[:], in1=csri[:], op=ALU.mult
                )
                cur = nxt
                if it < SINKHORN_ITER - 1:
                    rs2 = smallp.tile([128, PAIRS], F32, tag="rs")
                    nc.vector.reduce_sum(
                        rs2[:], nxt[:].rearrange("p (h j) -> p h j", h=2), axis=AX.X
                    )
                    rs = rs2

            # output: out[2h+v][i, j] = E[(v,i), (h,j)]; one DMA per
            # partition half v on separate queues
            ov = out_d[:].rearrange("(h v) i j -> v i h j", h=2, v=2)
            nc.sync.dma_start(
                ov[0], cur[0:64].rearrange("p (h j) -> p h j", h=2)
            )
            nc.scalar.dma_start(
                ov[1], cur[64:128].rearrange("p (h j) -> p h j", h=2)
            )

    _preload_act_table(nc)
    nc.compile()
    return nc


# act_info.json act_func_sets index of natural_log_exp_and_others, the one
# table that serves Ln AND Exp (and Relu/Copy). Pre-loading it up front
# makes Bacc's membership-based fixpoint skip every per-activation
# ACT_TABLE_LOAD (1.28us each), two of which would land on the critical
# tail between pair-1's Ln and Exp.
ACT_SET_LN_EXP = 6


def _preload_act_table(nc, set_id=ACT_SET_LN_EXP):
    load = mybir.InstLoadActFuncSet(
        name=nc.get_next_instruction_name(), act_func_set_id=set_id, ins=[], outs=[]
    )
    seen_act_engine = False
    for blk in nc.main_func.blocks:
        for idx, inst in enumerate(blk.instructions):
            eng = getattr(inst, "engine", None)
            if eng != mybir.EngineType.Activation:
                continue
            # skip the framework preamble (branches/barriers); insert at the
            # first Activation-engine instruction of the kernel body
            if isinstance(
                inst,
                (
                    mybir.InstDMACopy,
                    mybir.InstActivation,
                    mybir.InstLoadActFuncSet,
                ),
            ):
                load.engine = eng
                nc.register_instruction(load)
                blk.instructions.insert(idx, load)
                return
            seen_act_engine = True
    raise AssertionError("no activation-engine body instruction found")


_NC = None


def _get_program():
    global _NC
    if _NC is None:
        _NC = _build_program()
    return _NC


def _stack_pairs(a):
    # [4, X, T] (bh-major) -> [128=(v,X), pair, T] with bh = 2*pair + v
    x, t = a.shape[1], a.shape[2]
    return np.ascontiguousarray(
        a.reshape(PAIRS, 2, x, t).transpose(1, 2, 0, 3).reshape(2 * x, PAIRS, t)
    )


def _make_in_maps(inputs):
    q = np.asarray(inputs["q"], dtype=np.float32).astype(ml_dtypes.bfloat16)
    k = np.asarray(inputs["k"], dtype=np.float32).astype(ml_dtypes.bfloat16)
    qpe = np.asarray(inputs["q_pos_emb"], dtype=np.float32)
    kpe = np.asarray(inputs["k_pos_emb"], dtype=np.float32)
    g = np.ascontiguousarray(inputs["gumbel"], dtype=np.float32)

    b = BH // HEADS
    qpos = np.broadcast_to(qpe, (b, HEADS, BUCKETS, DIM)).reshape(BH, BUCKETS, DIM)
    kpos = np.broadcast_to(kpe, (b, HEADS, BUCKETS, DIM)).reshape(BH, BUCKETS, DIM)
    eye = np.eye(128, dtype=np.float32)
    eyeb = (np.eye(128, dtype=np.float32) / 128.0).astype(ml_dtypes.bfloat16)
    wones = np.kron(np.eye(2, dtype=np.float32), np.ones((64, 64), np.float32))

    in_maps = []
    for c in range(N_CORES):
        sl = slice(NBH * c, NBH * (c + 1))
        in_maps.append(
            {
                "q": np.ascontiguousarray(q[sl]),
                "k": np.ascontiguousarray(k[sl]),
                "posq": _stack_pairs(qpos[sl]),
                "posk": _stack_pairs(kpos[sl]),
                "gum": _stack_pairs(g[sl]),
                "eyeb": eyeb,
                "eye": eye,
                "wones": wones,
            }
        )
    return in_maps


def run(inputs, trace=False):
    nc = _get_program()
    in_maps = _make_in_maps(inputs)
    res = run_bass_kernel_spmd(
        nc, in_maps, core_ids=list(range(N_CORES)), trace=trace
    )
    out = np.concatenate(
        [res.results[c]["out"] for c in range(N_CORES)], axis=0
    ).astype(np.float32)
    return out, res


def kernel(**inputs) -> np.ndarray:
    out, _ = run(inputs, trace=False)
    return out
